# revision 59
# baseline (speedup 1.0000x reference)
"""Multi-head attention on 8 Trainium2 NeuronCores (bf16, flipped av,
software-pipelined unit stream).

Problem: x[4, 2048, 1024], 16 heads x 64 dim.
  qkv = x @ w_qkv; attn = softmax(q k^T / 8); out = (attn v) @ w_out + b_out

Sharding: 8 cores = 4 batches x 2 head-groups (8 heads each).
Each core computes a partial out-projection over its 8 heads' dims;
host sums the two partials per batch and adds the bias.

Per-core schedule (all matmuls bf16 = 1 cycle/row; PSUM accumulates fp32):
  The work is a stream of 16 units (head-pair x i-chunk). Unit u's
  scores+exp run interleaved with unit u-1's av matmuls, buffered through
  an 8-tag deep ex pool, so ScalarE (128 x [128,2048] exp, ~242 us) and
  PE (~279 us) both stay saturated:
  - 1A: x chunks resident; only pair-0's qT/kT (m0,m4) + v projected;
    unit-0 scores/exp/av staircased live, unit-1 exps deep-buffered.
  - cascade units 1..15: av(u) + scores(u+1) per 2-jt step, with the six
    remaining qkT m-tiles (units 1-11) and the out-projection of ready
    i-chunks (units 13+) drip-fed between steps.
  Flipped av: ex[j, i-128] stationary, [v_h|ones] 65-col moving ->
  av[i, 65] accumulated over j; DVE reciprocal of the sums column,
  gpsimd scale to bf16, PE transpose (identity) to aoT[vd, i].
"""

import numpy as np

import concourse.bacc as bacc
import concourse.mybir as mybir
import concourse.tile as tile
from concourse.bass_utils import run_bass_kernel_spmd

F32 = mybir.dt.float32
BF16 = mybir.dt.bfloat16
AF = mybir.ActivationFunctionType
NPBF16 = mybir.dt.np(BF16)

B = 4          # batch
N = 2048       # sequence
DM = 1024      # model dim
NH = 16        # heads
DH = 64        # head dim
G = 2          # head groups (cores per batch)
HPC = NH // G  # heads per core = 8
CW = DH * HPC  # per-core qkv column width = 512

NCH = 512      # phase-1 x^T column chunk
ICH = 512      # phase-2 i (query) chunk per pair

KT = DM // 128      # 8 contraction tiles over d
MT = 2 * CW // 128  # 8 c-tiles for q|k
NJT = N // 128      # 16 j tiles
NIC = N // ICH      # 4 i chunks
NBK = ICH // 128    # 4 i blocks per chunk
NP = HPC // 2       # 4 head pairs
NU = NP * NIC       # 16 stream units
JTP = NJT // 2      # 8 two-jt steps per unit


def unit_pi(u):
    """Stream order: pairs 0,1 then pair 2 (all ic), then pair 3."""
    return (u // NIC, u % NIC)


def build_nc(reps=1):
    nc = bacc.Bacc(None, target_bir_lowering=False, debug=False)

    xT = nc.declare_dram_parameter("xT", [N // NCH, 128, KT * NCH], BF16,
                                   isOutput=False)
    wqk = nc.declare_dram_parameter("wqk", [DM, 2 * CW], BF16, isOutput=False)
    wv = nc.declare_dram_parameter("wv", [DM, CW], BF16, isOutput=False)
    wo = nc.declare_dram_parameter("wo", [CW, DM], BF16, isOutput=False)
    ident = nc.declare_dram_parameter("ident", [128, 128], BF16, isOutput=False)
    out = nc.declare_dram_parameter("out", [N, DM], F32, isOutput=True)

    with tile.TileContext(nc) as tc:
        with (
            tc.tile_pool(name="cpool", bufs=1) as cpool,
            # 8 PSUM banks: "s" 1x[128,2048] scores (4), avE/avO 1 each,
            # "p1" 2x[128,512] projections/out-proj
            tc.tile_pool(name="psS", bufs=2, space="PSUM") as psS,
            tc.tile_pool(name="psAv", bufs=1, space="PSUM") as psAv,
            tc.tile_pool(name="psC", bufs=2, space="PSUM") as psC,
            tc.tile_pool(name="epool", bufs=3) as epool,
            tc.tile_pool(name="edeep", bufs=1) as edeep,
            tc.tile_pool(name="npool", bufs=2) as npool,
            tc.tile_pool(name="w1pool", bufs=1) as w1pool,
            tc.tile_pool(name="xpool", bufs=1) as xpool,
            tc.tile_pool(name="lpool", bufs=1) as lpool,
        ):
          id_t = cpool.tile([128, 128], BF16, name="id_t")
          nc.gpsimd.dma_start(id_t[:], ident[:, :])

          def make_tiles():
            return dict(
                qkT=[cpool.tile([128, N], BF16, name=f"qkT{m}")
                     for m in range(MT)],
                v=[cpool.tile([128, HPC * (DH + 1)], BF16, name=f"v{j}")
                   for j in range(NJT)],
                wqk=[w1pool.tile([128, 2 * CW], BF16, name=f"wqk{k}")
                     for k in range(KT)],
                wv=[w1pool.tile([128, CW], BF16, name=f"wv{k}")
                    for k in range(KT)],
                x=[xpool.tile([128, KT * NCH], BF16, name=f"x{c}", tag=f"x{c}")
                   for c in range(N // NCH)],
                aoT=[lpool.tile([128, N], BF16, name=f"aoT{c}", tag=f"aoT{c}")
                     for c in range(NP)],
                wo=[lpool.tile([128, DM], BF16, name=f"wo{c}", tag=f"wo{c}")
                    for c in range(NP)],
            )

          def emit_dmas(R, ch=None):
            """Input DMAs; ch=None emits x for all chunks + all weights."""
            half_el = KT * NCH // 2
            for c in ([ch] if ch is not None else range(N // NCH)):
                nc.sync.dma_start(R["x"][c][:, 0:half_el], xT[c][:, 0:half_el])
                nc.sync.dma_start(R["x"][c][:, half_el:], xT[c][:, half_el:])
                if c == 0:
                    for k in range(KT):
                        nc.gpsimd.dma_start(R["wqk"][k][:],
                                            wqk[k * 128:(k + 1) * 128, :])
                    for k in range(KT):
                        nc.gpsimd.dma_start(R["wv"][k][:],
                                            wv[k * 128:(k + 1) * 128, :])
                    for c2 in range(NP):
                        nc.gpsimd.dma_start(R["wo"][c2][:],
                                            wo[c2 * 128:(c2 + 1) * 128, :])

          carry = None   # {"R": tiles, "pre0": n} prepared by previous rep
          ex_of = {}     # (u, jt) -> ex tile (popped by av); persists reps
          for _rep in range(reps):
            if carry is not None:
                R, pre0, pre1 = carry["R"], carry["pre0"], carry["pre1"]
            else:
                R, pre0, pre1 = make_tiles(), 0, 0
            qkT_t, v_t = R["qkT"], R["v"]
            wqk_t, wv_t, x_t = R["wqk"], R["wv"], R["x"]
            aoT_t, wo_t = R["aoT"], R["wo"]

            # ---------- attention building blocks ----------
            def scores_exp(u, jt, deep, Rq=None):
                """One j-tile of both heads' scoresT + [128,1024] exp."""
                p, ic = unit_pi(u)
                qk = (Rq or R)["qkT"]
                qt, kt = qk[p], qk[MT // 2 + p]
                isl = slice(ic * ICH, (ic + 1) * ICH)
                s_ps = psS.tile([128, 2 * ICH], F32, name="s_ps", tag="s")
                for half in range(2):
                    off = half * DH
                    nc.tensor.matmul(
                        s_ps[:, half * ICH:(half + 1) * ICH],
                        kt[off:off + DH, jt * 128:(jt + 1) * 128],
                        qt[off:off + DH, isl],
                        start=True, stop=True,
                    )
                if deep:
                    # alternate tag sets by unit parity: the exp stream may
                    # run up to two units ahead of the av stream
                    ex = edeep.tile([128, 2 * ICH], BF16, name=f"exd{jt}",
                                    tag=f"exd{u % 2}_{jt}")
                else:
                    ex = epool.tile([128, 2 * ICH], BF16, name="ex", tag="ex")
                nc.scalar.activation(ex[:], s_ps[:], AF.Exp, scale=0.125)
                ex_of[(u, jt)] = ex

            def av_jt(u, jt, av2):
                """8 flipped-av matmuls for one j-tile of unit u."""
                p, ic = unit_pi(u)
                ex = ex_of.pop((u, jt))
                first, last = jt == 0, jt == NJT - 1
                for half in range(2):
                    h = 2 * p + half
                    av3 = av2[half]
                    for k in range(NBK):
                        nc.tensor.matmul(
                            av3[:, k, :],
                            ex[:, half * ICH + k * 128:half * ICH + (k + 1) * 128],
                            v_t[jt][:, h * (DH + 1):(h + 1) * (DH + 1)],
                            start=(first and k == 0), stop=(last and k == NBK - 1),
                            skip_group_check=True,
                        )

            def norm_scale(u, av2):
                """DVE part of normalize: reciprocal + scale to bf16 SBUF.
                Frees the av accumulator banks for the next unit."""
                rc = npool.tile([128, 2 * NBK], F32, name="rc", tag="rc")
                for half in range(2):
                    nc.vector.reciprocal(rc[:, half * NBK:(half + 1) * NBK],
                                         av2[half][:, :, DH:DH + 1])
                avn = npool.tile([128, NBK, 2 * DH], BF16, name="avn", tag="avn")
                for k in range(NBK):
                    for half in range(2):
                        # DVE, not gpsimd: GPSIMD cannot access PSUM
                        nc.vector.tensor_scalar_mul(
                            avn[:, k, half * DH:(half + 1) * DH],
                            av2[half][:, k, 0:DH],
                            rc[:, half * NBK + k:half * NBK + k + 1],
                        )
                return avn

            def norm_flush(u, avn, per_iblk=None):
                """PE transposes + copy-out (avT staged through psC)."""
                p, ic = unit_pi(u)
                avT = psC.tile([128, NBK, 128], BF16, name="avT", tag="p1")
                for k in range(NBK):
                    nc.tensor.matmul(
                        avT[:, k, :], avn[:, k, :], id_t[:],
                        is_transpose=True,
                        start=(k == 0), stop=(k == NBK - 1),
                        skip_group_check=True,
                    )
                    if per_iblk is not None:
                        nc.vector.tensor_copy(
                            aoT_t[p][:, (ic * NBK + k) * 128:
                                     (ic * NBK + k + 1) * 128],
                            avT[:, k, :],
                        )
                        per_iblk(k)
                if per_iblk is None:
                    nc.vector.tensor_copy(
                        aoT_t[p][:, ic * ICH:(ic + 1) * ICH],
                        avT.rearrange("p q c -> p (q c)")[:],
                    )

            def alloc_av2():
                return [psAv.tile([128, NBK, DH + 1], F32, name=f"av{h}",
                                  tag=("avE", "avO")[h]) for h in range(2)]

            # ---------- projection blocks ----------
            def proj_m(m, ch):
                pq = psC.tile([128, NCH], F32, name="pq", tag="p1")
                for k in range(KT):
                    nc.tensor.matmul(
                        pq[:],
                        wqk_t[k][:, m * 128:(m + 1) * 128],
                        x_t[ch][:, k * NCH:(k + 1) * NCH],
                        start=(k == 0), stop=(k == KT - 1),
                    )
                nc.vector.tensor_copy(
                    qkT_t[m][:, ch * NCH:(ch + 1) * NCH], pq[:])

            def proj_v(j):
                ch, mt = divmod(j, NCH // 128)
                pv = psC.tile([128, CW], F32, name="pv", tag="p1")
                for k in range(KT):
                    nc.tensor.matmul(
                        pv[:],
                        x_t[ch][:, k * NCH + mt * 128:k * NCH + (mt + 1) * 128],
                        wv_t[k][:],
                        start=(k == 0), stop=(k == KT - 1),
                    )
                v3 = v_t[j].rearrange("p (q c) -> p q c", c=DH + 1)
                nc.vector.tensor_copy(v3[:, :, 0:DH],
                                      pv[:].rearrange("p (l c) -> p l c", c=DH))
                nc.any.memset(v3[:, :, DH:DH + 1], 1.0)

            def p1b_gen():
                for m in (1, 5, 2, 6, 3, 7):
                    for ch in range(N // NCH):
                        pq = psC.tile([128, NCH], F32, name="pq", tag="p1")
                        for k in range(KT):
                            yield nc.tensor.matmul(
                                pq[:],
                                wqk_t[k][:, m * 128:(m + 1) * 128],
                                x_t[ch][:, k * NCH:(k + 1) * NCH],
                                start=(k == 0), stop=(k == KT - 1),
                            )
                        nc.vector.tensor_copy(
                            qkT_t[m][:, ch * NCH:(ch + 1) * NCH], pq[:])
                while True:
                    yield None

            p1b = p1b_gen()

            def p1b_tick(n):
                for _ in range(n):
                    next(p1b)

            def phase3_group(nt, h, tail=False):
                po = psC.tile([128, 512], F32, name="po", tag="p1")
                for c in range(NP):
                    nc.tensor.matmul(
                        po[:],
                        aoT_t[c][:, nt * 128:(nt + 1) * 128],
                        wo_t[c][:, h * 512:(h + 1) * 512],
                        start=(c == 0), stop=(c == NP - 1),
                    )
                # always DVE: a ScalarE copy here would make ScalarE's
                # in-order queue wait on the PE tail, blocking the next
                # rep's exps behind it
                os_ = lpool.tile([128, 512], F32, name="os", tag="os", bufs=3)
                nc.vector.tensor_copy(os_[:], po[:])
                nc.gpsimd.dma_start(
                    out[nt * 128:(nt + 1) * 128, h * 512:(h + 1) * 512],
                    os_[:],
                )

            # ---------------- phase 1A ----------------
            av2_u0 = alloc_av2()
            sc0 = pre0   # next jt scored for unit 0 (pre-run at prev tail)
            av0 = 0      # next jt av'd for unit 0
            sc1 = pre1   # next jt scored (deep) for unit 1
            sc2 = [0]  # next jt scored (deep) for unit 2

            def drain_1a(ch, v_done):
                nonlocal sc0, av0, sc1
                jt_lim = min(4 * (ch + 1), NJT)
                while True:
                    progress = False
                    if sc0 < jt_lim and sc0 - av0 < 3:
                        scores_exp(0, sc0, deep=False)
                        sc0 += 1
                        progress = True
                    while av0 < min(sc0, v_done):
                        av_jt(0, av0, av2_u0)
                        av0 += 1
                        progress = True
                    if ch >= 1 and sc1 < min(jt_lim, 2 * v_done + 2):
                        scores_exp(1, sc1, deep=True)
                        sc1 += 1
                        progress = True
                    # u2 shares the set-0 deep tags with u0's (pre-run)
                    # exps: only reuse a tag once u0's av has consumed it
                    if ch >= 2 and sc2[0] < min(jt_lim, av0):
                        scores_exp(2, sc2[0], deep=True)
                        sc2[0] += 1
                        progress = True
                    if not progress:
                        return

            if carry is None:
                # prologue: DMAs + m0/m4 inline (first rep only)
                for ch in range(N // NCH):
                    emit_dmas(R, ch)
                    proj_m(0, ch)
                    proj_m(MT // 2, ch)
                    drain_1a(ch, 4 * ch)  # scores may precede this chunk's v
                    for mt in range(NCH // 128):
                        j = ch * (NCH // 128) + mt
                        proj_v(j)
                        drain_1a(ch, j + 1)
            else:
                # steady state: x/weights + m0/m4 already emitted by the
                # previous rep's late cascade -> exps flow immediately
                for j in range(NJT):
                    proj_v(j)
                    drain_1a(N // NCH - 1, j + 1)

            assert sc0 == NJT and av0 == NJT and sc1 == NJT, (sc0, av0, sc1)
            pend_flush = (0, norm_scale(0, av2_u0))

            # ---------------- cascade: units 1..15 ----------------
            # next-rep m0/m4 drip generator (filled into units 14-15)
            nextR = None
            if _rep + 1 < reps:
                nextR = make_tiles()

            def ngen_f():
                if nextR is None:
                    while True:
                        yield None
                for m in (0, MT // 2):
                    for ch in range(N // NCH):
                        pq = psC.tile([128, NCH], F32, name="pq", tag="p1")
                        for k in range(KT):
                            yield nc.tensor.matmul(
                                pq[:],
                                nextR["wqk"][k][:, m * 128:(m + 1) * 128],
                                nextR["x"][ch][:, k * NCH:(k + 1) * NCH],
                                start=(k == 0), stop=(k == KT - 1),
                            )
                        nc.vector.tensor_copy(
                            nextR["qkT"][m][:, ch * NCH:(ch + 1) * NCH], pq[:])
                while True:
                    yield None

            ngen = ngen_f()
            scored = [NJT, NJT, sc2[0]] + [0] * (NU - 3)
            p3_groups = []
            for u in range(1, NU):
                av2 = alloc_av2()
                last = u == NU - 1
                if u == 12 and nextR is not None:
                    emit_dmas(nextR)   # prefetch next rep's inputs
                for jt in range(NJT):
                    if scored[u] <= jt:   # self catch-up (shouldn't happen)
                        scores_exp(u, jt, deep=True)
                        scored[u] += 1
                    av_jt(u, jt, av2)
                    if u + 1 < NU and scored[u + 1] <= jt:
                        scores_exp(u + 1, jt, deep=True)
                        scored[u + 1] += 1
                    if jt == 1 and pend_flush is not None:
                        # previous unit's transposes, off the critical chain
                        norm_flush(*pend_flush)
                        pend_flush = None
                    # fillers
                    if u <= 3:
                        p1b_tick(2 if jt % 2 else 1)
                    elif u <= 11:
                        p1b_tick(1)
                    if u == 12:
                        next(ngen)
                        if jt % 2 == 0:
                            next(ngen)
                    elif u >= 13:
                        next(ngen)
                    if u >= 13 and jt % 2 and p3_groups:
                        phase3_group(*p3_groups.pop(0))
                p, ic = unit_pi(u)
                avn = norm_scale(u, av2)
                if not last:
                    pend_flush = (u, avn)
                else:
                    # next rep's unit-0/unit-1 scores first: their exps
                    # front-run the PE tail below and the next 1A's
                    # v-projections, keeping ScalarE busy through the rep
                    # boundary (set-0/1 deep tags are free: this rep's
                    # unit-14/15 avs have consumed them)
                    pre_n = [0]
                    pre_n1 = [0]
                    if nextR is not None:
                        while pre_n[0] < NJT:
                            scores_exp(0, pre_n[0], deep=True, Rq=nextR)
                            pre_n[0] += 1
                        while pre_n1[0] < NJT:
                            scores_exp(1, pre_n1[0], deep=True, Rq=nextR)
                            pre_n1[0] += 1
                    # tail: iblk-granular transposes interleaved with the
                    # final i-chunk's out-projection groups
                    def tail_iblk(k):
                        for h in range(DM // 512):
                            phase3_group(ic * NBK + k, h, tail=True)
                    norm_flush(u, avn, per_iblk=tail_iblk)
                if p == NP - 1 and not last:   # pair-3 i-chunk -> out-proj ready
                    p3_groups += [(nt, h)
                                  for nt in range(ic * ICH // 128,
                                                  (ic + 1) * ICH // 128)
                                  for h in range(DM // 512)]
            p1b_tick(MT * KT * (N // NCH))
            while p3_groups:
                phase3_group(*p3_groups.pop(0), tail=True)
            for _ in range(2 * KT * (N // NCH)):   # flush next-rep m0/m4
                next(ngen)
            if nextR is not None:
                carry = {"R": nextR, "pre0": pre_n[0], "pre1": pre_n1[0]}
            else:
                carry = None

    nc.finalize()
    return nc


def core_in_maps(x, w_qkv, w_out):
    """Per-core input dict list (cores 0-7 = 4 batches x 2 head groups)."""
    x = np.ascontiguousarray(x, dtype=np.float32)
    w_qkv = np.asarray(w_qkv, dtype=np.float32)
    w_out = np.asarray(w_out, dtype=np.float32)
    id128 = np.eye(128, dtype=NPBF16)
    in_maps = []
    xp_cache = {}
    for core in range(8):
        b, g = divmod(core, 2)
        if b not in xp_cache:
            xp_cache[b] = np.ascontiguousarray(
                x[b].T.reshape(KT, 128, N // NCH, NCH).transpose(2, 1, 0, 3)
                .astype(NPBF16)
            ).reshape(N // NCH, 128, KT * NCH)
        xTb = xp_cache[b]
        wq = w_qkv[:, g * CW:(g + 1) * CW]
        wk = w_qkv[:, DM + g * CW:DM + (g + 1) * CW]
        wv_ = w_qkv[:, 2 * DM + g * CW:2 * DM + (g + 1) * CW]
        in_maps.append({
            "xT": xTb,
            "wqk": np.ascontiguousarray(
                np.concatenate([wq, wk], axis=1).astype(NPBF16)),
            "wv": np.ascontiguousarray(wv_.astype(NPBF16)),
            "wo": np.ascontiguousarray(
                w_out[g * CW:(g + 1) * CW, :].astype(NPBF16)),
            "ident": id128,
        })
    return in_maps


_NC_CACHE = {}


def _get_nc():
    if "nc" not in _NC_CACHE:
        _NC_CACHE["nc"] = build_nc()
    return _NC_CACHE["nc"]


def kernel(x, w_qkv, w_out, b_out):
    b_out = np.asarray(b_out, dtype=np.float32)
    nc = _get_nc()
    in_maps = core_in_maps(x, w_qkv, w_out)
    res = run_bass_kernel_spmd(nc, in_maps, core_ids=list(range(8)))
    _NC_CACHE["last_result"] = res
    out = np.empty((B, N, DM), np.float32)
    for b in range(B):
        out[b] = res.results[2 * b]["out"] + res.results[2 * b + 1]["out"] + b_out
    return out


# revision 60
# speedup vs baseline: 1.1450x; 1.1450x over previous
"""Multi-head attention on 8 Trainium2 NeuronCores (bf16, flipped av,
software-pipelined unit stream with cross-rep pipelining).

Problem: x[4, 2048, 1024], 16 heads x 64 dim.
  qkv = x @ w_qkv; attn = softmax(q k^T / 8); out = (attn v) @ w_out + b_out

Sharding: 8 cores = 4 batches x 2 head-groups (8 heads each).
Each core computes a partial out-projection over its 8 heads' dims;
host sums the two partials per batch and adds the bias.

Per-core schedule (all matmuls bf16 = 1 cycle/row; PSUM accumulates fp32):
  The work is a stream of 16 units (head-pair x i-chunk). Unit u's
  scores+exp run interleaved with unit u-1's av matmuls through two
  16-tag deep ex-tile sets (alternating by unit parity, so the exp
  stream may run two units ahead), keeping ScalarE (256 x [128,1024]
  exp, ~266 us busy) and PE (~279 us busy) both near saturation:
  - 1A: x chunks resident; only pair-0's qT/kT (m0,m4) + v projected;
    unit-0 scores/exp/av staircased live, unit-1/2 exps deep-buffered.
  - cascade units 1..15: av(u) + scores(u+1) per j-tile step, with the
    six remaining qkT m-tiles (units 1-11), the out-projection of ready
    i-chunks (units 13+), and the NEXT rep's m0/m4 (units 12-15)
    drip-fed between steps.
  - rep boundary: the next rep's input DMAs prefetch at unit 12, and its
    unit-0/1 scores+exps are emitted at this rep's tail so ScalarE stays
    busy through the boundary; steady-state reps then skip straight to
    v-projections. PSUM: 4 banks scores (2x2), 2 av accumulators, 2
    projection (shared with transpose staging).
  Flipped av: ex[j, i-128] stationary, [v_h|ones] 65-col moving ->
  av[i, 65] accumulated over j (bank-packed accumulation groups); DVE
  reciprocal of the sums column + scale to bf16, PE transpose (identity)
  to aoT[vd, i], deferred one unit off the accumulator-bank chain.
  Measured on the 8-core axon TRN2: rel err 2.73e-3; cost-model marginal
  rep 294 us (single-shot 315 us); slope-method hw readings 187-346 us
  across sessions depending on tunnel congestion.
"""

import numpy as np

import concourse.bacc as bacc
import concourse.mybir as mybir
import concourse.tile as tile
from concourse.bass_utils import run_bass_kernel_spmd

F32 = mybir.dt.float32
BF16 = mybir.dt.bfloat16
AF = mybir.ActivationFunctionType
NPBF16 = mybir.dt.np(BF16)

B = 4          # batch
N = 2048       # sequence
DM = 1024      # model dim
NH = 16        # heads
DH = 64        # head dim
G = 2          # head groups (cores per batch)
HPC = NH // G  # heads per core = 8
CW = DH * HPC  # per-core qkv column width = 512

NCH = 512      # phase-1 x^T column chunk
ICH = 512      # phase-2 i (query) chunk per pair

KT = DM // 128      # 8 contraction tiles over d
MT = 2 * CW // 128  # 8 c-tiles for q|k
NJT = N // 128      # 16 j tiles
NIC = N // ICH      # 4 i chunks
NBK = ICH // 128    # 4 i blocks per chunk
NP = HPC // 2       # 4 head pairs
NU = NP * NIC       # 16 stream units
JTP = NJT // 2      # 8 two-jt steps per unit


def unit_pi(u):
    """Stream order: pairs 0,1 then pair 2 (all ic), then pair 3."""
    return (u // NIC, u % NIC)


def build_nc(reps=1):
    nc = bacc.Bacc(None, target_bir_lowering=False, debug=False)

    xT = nc.declare_dram_parameter("xT", [N // NCH, 128, KT * NCH], BF16,
                                   isOutput=False)
    wqk = nc.declare_dram_parameter("wqk", [DM, 2 * CW], BF16, isOutput=False)
    wv = nc.declare_dram_parameter("wv", [DM, CW], BF16, isOutput=False)
    wo = nc.declare_dram_parameter("wo", [CW, DM], BF16, isOutput=False)
    ident = nc.declare_dram_parameter("ident", [128, 128], BF16, isOutput=False)
    out = nc.declare_dram_parameter("out", [N, DM], F32, isOutput=True)

    with tile.TileContext(nc) as tc:
        with (
            tc.tile_pool(name="cpool", bufs=1) as cpool,
            # 8 PSUM banks: "s" 1x[128,2048] scores (4), avE/avO 1 each,
            # "p1" 2x[128,512] projections/out-proj
            tc.tile_pool(name="psS", bufs=2, space="PSUM") as psS,
            tc.tile_pool(name="psAv", bufs=1, space="PSUM") as psAv,
            tc.tile_pool(name="psC", bufs=2, space="PSUM") as psC,
            tc.tile_pool(name="epool", bufs=3) as epool,
            tc.tile_pool(name="edeep", bufs=1) as edeep,
            tc.tile_pool(name="npool", bufs=2) as npool,
            tc.tile_pool(name="w1pool", bufs=1) as w1pool,
            tc.tile_pool(name="xpool", bufs=1) as xpool,
            tc.tile_pool(name="lpool", bufs=1) as lpool,
        ):
          id_t = cpool.tile([128, 128], BF16, name="id_t")
          nc.gpsimd.dma_start(id_t[:], ident[:, :])

          def make_tiles():
            return dict(
                qkT=[cpool.tile([128, N], BF16, name=f"qkT{m}")
                     for m in range(MT)],
                v=[cpool.tile([128, HPC * (DH + 1)], BF16, name=f"v{j}")
                   for j in range(NJT)],
                wqk=[w1pool.tile([128, 2 * CW], BF16, name=f"wqk{k}")
                     for k in range(KT)],
                wv=[w1pool.tile([128, CW], BF16, name=f"wv{k}")
                    for k in range(KT)],
                x=[xpool.tile([128, KT * NCH], BF16, name=f"x{c}", tag=f"x{c}")
                   for c in range(N // NCH)],
                aoT=[lpool.tile([128, N], BF16, name=f"aoT{c}", tag=f"aoT{c}")
                     for c in range(NP)],
                wo=[lpool.tile([128, DM], BF16, name=f"wo{c}", tag=f"wo{c}")
                    for c in range(NP)],
            )

          def emit_dmas(R, ch=None):
            """Input DMAs; ch=None emits x for all chunks + all weights."""
            half_el = KT * NCH // 2
            for c in ([ch] if ch is not None else range(N // NCH)):
                nc.sync.dma_start(R["x"][c][:, 0:half_el], xT[c][:, 0:half_el])
                nc.sync.dma_start(R["x"][c][:, half_el:], xT[c][:, half_el:])
                if c == 0:
                    for k in range(KT):
                        nc.gpsimd.dma_start(R["wqk"][k][:],
                                            wqk[k * 128:(k + 1) * 128, :])
                    for k in range(KT):
                        nc.gpsimd.dma_start(R["wv"][k][:],
                                            wv[k * 128:(k + 1) * 128, :])
                    for c2 in range(NP):
                        nc.gpsimd.dma_start(R["wo"][c2][:],
                                            wo[c2 * 128:(c2 + 1) * 128, :])

          carry = None   # {"R": tiles, "pre0": n} prepared by previous rep
          ex_of = {}     # (u, jt) -> ex tile (popped by av); persists reps
          for _rep in range(reps):
            if carry is not None:
                R, pre0, pre1 = carry["R"], carry["pre0"], carry["pre1"]
            else:
                R, pre0, pre1 = make_tiles(), 0, 0
            qkT_t, v_t = R["qkT"], R["v"]
            wqk_t, wv_t, x_t = R["wqk"], R["wv"], R["x"]
            aoT_t, wo_t = R["aoT"], R["wo"]

            # ---------- attention building blocks ----------
            def scores_exp(u, jt, deep, Rq=None):
                """One j-tile of both heads' scoresT + [128,1024] exp."""
                p, ic = unit_pi(u)
                qk = (Rq or R)["qkT"]
                qt, kt = qk[p], qk[MT // 2 + p]
                isl = slice(ic * ICH, (ic + 1) * ICH)
                s_ps = psS.tile([128, 2 * ICH], F32, name="s_ps", tag="s")
                for half in range(2):
                    off = half * DH
                    nc.tensor.matmul(
                        s_ps[:, half * ICH:(half + 1) * ICH],
                        kt[off:off + DH, jt * 128:(jt + 1) * 128],
                        qt[off:off + DH, isl],
                        start=True, stop=True,
                    )
                if deep:
                    # alternate tag sets by unit parity: the exp stream may
                    # run up to two units ahead of the av stream
                    ex = edeep.tile([128, 2 * ICH], BF16, name=f"exd{jt}",
                                    tag=f"exd{u % 2}_{jt}")
                else:
                    ex = epool.tile([128, 2 * ICH], BF16, name="ex", tag="ex")
                nc.scalar.activation(ex[:], s_ps[:], AF.Exp, scale=0.125)
                ex_of[(u, jt)] = ex

            def av_jt(u, jt, av2):
                """8 flipped-av matmuls for one j-tile of unit u."""
                p, ic = unit_pi(u)
                ex = ex_of.pop((u, jt))
                first, last = jt == 0, jt == NJT - 1
                for half in range(2):
                    h = 2 * p + half
                    av3 = av2[half]
                    for k in range(NBK):
                        nc.tensor.matmul(
                            av3[:, k, :],
                            ex[:, half * ICH + k * 128:half * ICH + (k + 1) * 128],
                            v_t[jt][:, h * (DH + 1):(h + 1) * (DH + 1)],
                            start=(first and k == 0), stop=(last and k == NBK - 1),
                            skip_group_check=True,
                        )

            def norm_scale(u, av2):
                """DVE part of normalize: reciprocal + scale to bf16 SBUF.
                Frees the av accumulator banks for the next unit."""
                rc = npool.tile([128, 2 * NBK], F32, name="rc", tag="rc")
                for half in range(2):
                    nc.vector.reciprocal(rc[:, half * NBK:(half + 1) * NBK],
                                         av2[half][:, :, DH:DH + 1])
                avn = npool.tile([128, NBK, 2 * DH], BF16, name="avn", tag="avn")
                for k in range(NBK):
                    for half in range(2):
                        # DVE, not gpsimd: GPSIMD cannot access PSUM
                        nc.vector.tensor_scalar_mul(
                            avn[:, k, half * DH:(half + 1) * DH],
                            av2[half][:, k, 0:DH],
                            rc[:, half * NBK + k:half * NBK + k + 1],
                        )
                return avn

            def norm_flush(u, avn, per_iblk=None):
                """PE transposes + copy-out (avT staged through psC)."""
                p, ic = unit_pi(u)
                avT = psC.tile([128, NBK, 128], BF16, name="avT", tag="p1")
                for k in range(NBK):
                    nc.tensor.matmul(
                        avT[:, k, :], avn[:, k, :], id_t[:],
                        is_transpose=True,
                        start=(k == 0), stop=(k == NBK - 1),
                        skip_group_check=True,
                    )
                    if per_iblk is not None:
                        nc.vector.tensor_copy(
                            aoT_t[p][:, (ic * NBK + k) * 128:
                                     (ic * NBK + k + 1) * 128],
                            avT[:, k, :],
                        )
                        per_iblk(k)
                if per_iblk is None:
                    nc.vector.tensor_copy(
                        aoT_t[p][:, ic * ICH:(ic + 1) * ICH],
                        avT.rearrange("p q c -> p (q c)")[:],
                    )

            def alloc_av2():
                return [psAv.tile([128, NBK, DH + 1], F32, name=f"av{h}",
                                  tag=("avE", "avO")[h]) for h in range(2)]

            # ---------- projection blocks ----------
            def proj_m(m, ch):
                pq = psC.tile([128, NCH], F32, name="pq", tag="p1")
                for k in range(KT):
                    nc.tensor.matmul(
                        pq[:],
                        wqk_t[k][:, m * 128:(m + 1) * 128],
                        x_t[ch][:, k * NCH:(k + 1) * NCH],
                        start=(k == 0), stop=(k == KT - 1),
                    )
                nc.vector.tensor_copy(
                    qkT_t[m][:, ch * NCH:(ch + 1) * NCH], pq[:])

            def proj_v(j):
                ch, mt = divmod(j, NCH // 128)
                pv = psC.tile([128, CW], F32, name="pv", tag="p1")
                for k in range(KT):
                    nc.tensor.matmul(
                        pv[:],
                        x_t[ch][:, k * NCH + mt * 128:k * NCH + (mt + 1) * 128],
                        wv_t[k][:],
                        start=(k == 0), stop=(k == KT - 1),
                    )
                v3 = v_t[j].rearrange("p (q c) -> p q c", c=DH + 1)
                nc.vector.tensor_copy(v3[:, :, 0:DH],
                                      pv[:].rearrange("p (l c) -> p l c", c=DH))
                nc.any.memset(v3[:, :, DH:DH + 1], 1.0)

            def p1b_gen():
                for m in (1, 5, 2, 6, 3, 7):
                    for ch in range(N // NCH):
                        pq = psC.tile([128, NCH], F32, name="pq", tag="p1")
                        for k in range(KT):
                            yield nc.tensor.matmul(
                                pq[:],
                                wqk_t[k][:, m * 128:(m + 1) * 128],
                                x_t[ch][:, k * NCH:(k + 1) * NCH],
                                start=(k == 0), stop=(k == KT - 1),
                            )
                        nc.vector.tensor_copy(
                            qkT_t[m][:, ch * NCH:(ch + 1) * NCH], pq[:])
                while True:
                    yield None

            p1b = p1b_gen()

            def p1b_tick(n):
                for _ in range(n):
                    next(p1b)

            def phase3_group(nt, h, tail=False):
                po = psC.tile([128, 512], F32, name="po", tag="p1")
                for c in range(NP):
                    nc.tensor.matmul(
                        po[:],
                        aoT_t[c][:, nt * 128:(nt + 1) * 128],
                        wo_t[c][:, h * 512:(h + 1) * 512],
                        start=(c == 0), stop=(c == NP - 1),
                    )
                # always DVE: a ScalarE copy here would make ScalarE's
                # in-order queue wait on the PE tail, blocking the next
                # rep's exps behind it
                os_ = lpool.tile([128, 512], F32, name="os", tag="os", bufs=3)
                nc.vector.tensor_copy(os_[:], po[:])
                nc.gpsimd.dma_start(
                    out[nt * 128:(nt + 1) * 128, h * 512:(h + 1) * 512],
                    os_[:],
                )

            # ---------------- phase 1A ----------------
            av2_u0 = alloc_av2()
            sc0 = pre0   # next jt scored for unit 0 (pre-run at prev tail)
            av0 = 0      # next jt av'd for unit 0
            sc1 = pre1   # next jt scored (deep) for unit 1
            sc2 = [0]  # next jt scored (deep) for unit 2

            def drain_1a(ch, v_done):
                nonlocal sc0, av0, sc1
                jt_lim = min(4 * (ch + 1), NJT)
                while True:
                    progress = False
                    if sc0 < jt_lim and sc0 - av0 < 3:
                        scores_exp(0, sc0, deep=False)
                        sc0 += 1
                        progress = True
                    while av0 < min(sc0, v_done):
                        av_jt(0, av0, av2_u0)
                        av0 += 1
                        progress = True
                    if ch >= 1 and sc1 < min(jt_lim, 2 * v_done + 2):
                        scores_exp(1, sc1, deep=True)
                        sc1 += 1
                        progress = True
                    # u2 shares the set-0 deep tags with u0's (pre-run)
                    # exps: only reuse a tag once u0's av has consumed it
                    if ch >= 2 and sc2[0] < min(jt_lim, av0):
                        scores_exp(2, sc2[0], deep=True)
                        sc2[0] += 1
                        progress = True
                    if not progress:
                        return

            if carry is None:
                # prologue: DMAs + m0/m4 inline (first rep only)
                for ch in range(N // NCH):
                    emit_dmas(R, ch)
                    proj_m(0, ch)
                    proj_m(MT // 2, ch)
                    drain_1a(ch, 4 * ch)  # scores may precede this chunk's v
                    for mt in range(NCH // 128):
                        j = ch * (NCH // 128) + mt
                        proj_v(j)
                        drain_1a(ch, j + 1)
            else:
                # steady state: x/weights + m0/m4 already emitted by the
                # previous rep's late cascade -> exps flow immediately
                for j in range(NJT):
                    proj_v(j)
                    drain_1a(N // NCH - 1, j + 1)

            assert sc0 == NJT and av0 == NJT and sc1 == NJT, (sc0, av0, sc1)
            pend_flush = (0, norm_scale(0, av2_u0))

            # ---------------- cascade: units 1..15 ----------------
            # next-rep m0/m4 drip generator (filled into units 14-15)
            nextR = None
            if _rep + 1 < reps:
                nextR = make_tiles()

            def ngen_f():
                if nextR is None:
                    while True:
                        yield None
                for m in (0, MT // 2):
                    for ch in range(N // NCH):
                        pq = psC.tile([128, NCH], F32, name="pq", tag="p1")
                        for k in range(KT):
                            yield nc.tensor.matmul(
                                pq[:],
                                nextR["wqk"][k][:, m * 128:(m + 1) * 128],
                                nextR["x"][ch][:, k * NCH:(k + 1) * NCH],
                                start=(k == 0), stop=(k == KT - 1),
                            )
                        nc.vector.tensor_copy(
                            nextR["qkT"][m][:, ch * NCH:(ch + 1) * NCH], pq[:])
                while True:
                    yield None

            ngen = ngen_f()
            scored = [NJT, NJT, sc2[0]] + [0] * (NU - 3)
            p3_groups = []
            for u in range(1, NU):
                av2 = alloc_av2()
                last = u == NU - 1
                if u == 12 and nextR is not None:
                    emit_dmas(nextR)   # prefetch next rep's inputs
                for jt in range(NJT):
                    if scored[u] <= jt:   # self catch-up (shouldn't happen)
                        scores_exp(u, jt, deep=True)
                        scored[u] += 1
                    av_jt(u, jt, av2)
                    if u + 1 < NU and scored[u + 1] <= jt:
                        scores_exp(u + 1, jt, deep=True)
                        scored[u + 1] += 1
                    if jt == 1 and pend_flush is not None:
                        # previous unit's transposes, off the critical chain
                        norm_flush(*pend_flush)
                        pend_flush = None
                    # fillers
                    if u <= 3:
                        p1b_tick(2 if jt % 2 else 1)
                    elif u <= 11:
                        p1b_tick(1)
                    if u == 12:
                        next(ngen)
                        if jt % 2 == 0:
                            next(ngen)
                    elif u >= 13:
                        next(ngen)
                    if u >= 13 and jt % 2 and p3_groups:
                        phase3_group(*p3_groups.pop(0))
                p, ic = unit_pi(u)
                avn = norm_scale(u, av2)
                if not last:
                    pend_flush = (u, avn)
                else:
                    # next rep's unit-0/unit-1 scores first: their exps
                    # front-run the PE tail below and the next 1A's
                    # v-projections, keeping ScalarE busy through the rep
                    # boundary (set-0/1 deep tags are free: this rep's
                    # unit-14/15 avs have consumed them)
                    pre_n = [0]
                    pre_n1 = [0]
                    if nextR is not None:
                        while pre_n[0] < NJT:
                            scores_exp(0, pre_n[0], deep=True, Rq=nextR)
                            pre_n[0] += 1
                        while pre_n1[0] < NJT:
                            scores_exp(1, pre_n1[0], deep=True, Rq=nextR)
                            pre_n1[0] += 1
                    # tail: iblk-granular transposes interleaved with the
                    # final i-chunk's out-projection groups
                    def tail_iblk(k):
                        for h in range(DM // 512):
                            phase3_group(ic * NBK + k, h, tail=True)
                    norm_flush(u, avn, per_iblk=tail_iblk)
                if p == NP - 1 and not last:   # pair-3 i-chunk -> out-proj ready
                    p3_groups += [(nt, h)
                                  for nt in range(ic * ICH // 128,
                                                  (ic + 1) * ICH // 128)
                                  for h in range(DM // 512)]
            p1b_tick(MT * KT * (N // NCH))
            while p3_groups:
                phase3_group(*p3_groups.pop(0), tail=True)
            for _ in range(2 * KT * (N // NCH)):   # flush next-rep m0/m4
                next(ngen)
            if nextR is not None:
                carry = {"R": nextR, "pre0": pre_n[0], "pre1": pre_n1[0]}
            else:
                carry = None

    nc.finalize()
    return nc


def core_in_maps(x, w_qkv, w_out):
    """Per-core input dict list (cores 0-7 = 4 batches x 2 head groups)."""
    x = np.ascontiguousarray(x, dtype=np.float32)
    w_qkv = np.asarray(w_qkv, dtype=np.float32)
    w_out = np.asarray(w_out, dtype=np.float32)
    id128 = np.eye(128, dtype=NPBF16)
    in_maps = []
    xp_cache = {}
    for core in range(8):
        b, g = divmod(core, 2)
        if b not in xp_cache:
            xp_cache[b] = np.ascontiguousarray(
                x[b].T.reshape(KT, 128, N // NCH, NCH).transpose(2, 1, 0, 3)
                .astype(NPBF16)
            ).reshape(N // NCH, 128, KT * NCH)
        xTb = xp_cache[b]
        wq = w_qkv[:, g * CW:(g + 1) * CW]
        wk = w_qkv[:, DM + g * CW:DM + (g + 1) * CW]
        wv_ = w_qkv[:, 2 * DM + g * CW:2 * DM + (g + 1) * CW]
        in_maps.append({
            "xT": xTb,
            "wqk": np.ascontiguousarray(
                np.concatenate([wq, wk], axis=1).astype(NPBF16)),
            "wv": np.ascontiguousarray(wv_.astype(NPBF16)),
            "wo": np.ascontiguousarray(
                w_out[g * CW:(g + 1) * CW, :].astype(NPBF16)),
            "ident": id128,
        })
    return in_maps


_NC_CACHE = {}


def _get_nc():
    if "nc" not in _NC_CACHE:
        _NC_CACHE["nc"] = build_nc()
    return _NC_CACHE["nc"]


def kernel(x, w_qkv, w_out, b_out):
    b_out = np.asarray(b_out, dtype=np.float32)
    nc = _get_nc()
    in_maps = core_in_maps(x, w_qkv, w_out)
    res = run_bass_kernel_spmd(nc, in_maps, core_ids=list(range(8)))
    _NC_CACHE["last_result"] = res
    out = np.empty((B, N, DM), np.float32)
    for b in range(B):
        out[b] = res.results[2 * b]["out"] + res.results[2 * b + 1]["out"] + b_out
    return out
